# revision 1
# baseline (speedup 1.0000x reference)
"""Trainium2 Bass kernel for nn_CompressiveMemory_57750130262084.

The reference computes (B=8, S=4096, DK=DV=1024):
    sigma  = elu(query) + 1                                  [B,S,DK]
    memory = einsum('bkd,bsv->bkv', swap(sigma), value)      [B,DK,DV]
    z_norm = sum_s sigma                                     [B,DK]
    out    = einsum('bsd,bkv->bsv', sigma, memory)
           / einsum('bsd,bk->bs',  sigma, z_norm)[..., None]

Every einsum uses disjoint summed subscripts, so each factorises into
outer products of independent reductions:
    memory[b,k,v]    = z_norm[b,k] * VS[b,v]      with VS[b,v] = sum_s value[b,s,v]
    retrieved[b,s,v] = rs[b,s] * Z[b] * VS[b,v]   with rs = rowsum(sigma), Z = sum_k z_norm
    denom[b,s]       = rs[b,s] * Z[b]
    out[b,s,v]       = VS[b,v]                    (exactly; query cancels)

So the kernel is a column-sum of `value` over S, broadcast over S.
Sharding: data-parallel over batch, one NeuronCore per batch element.
Per-core work: read 16 MB, reduce 4096 rows -> 1 row, write 16 MB;
memory-bound at the shared-HBM per-NC limit (~330-360 GB/s).

Schedule per core:
  - input as HWDGE DMAs of descending size (chunks of 128 rows x 1024
    cols). Chunk reductions are split ~2:1 between the DVE (fp32
    tensor_add chain into acc, ~1.23 us/chunk, capped at 1x mode) and
    the PE (PSUM-accumulating ones[128,128]^T @ chunk, 2 HW passes per
    f32 N=512 bank, ~2.2 us/chunk) so both trail the DMA stream. The
    PE path partition-reduces AND broadcasts the colsum to all 128
    partitions in the same op; the DVE accumulator is folded into the
    same PSUM banks mid-stream, and the final chunks are PE-owned so
    the critical tail after the last input byte is ~2 passes + copy.
  - PSUM -> SBUF copy in halves (DVE + ACT); output DMAs use a step-0
    (broadcast) source AP to fan the single [128,1024] colsum tile out
    to all 4096 rows (faster than materialized replicas, measured).
"""

import numpy as np

B, S, D = 8, 4096, 1024
P = 128                 # SBUF partitions
N_CHUNK = S // P        # 32 row-chunks of 128 rows
IN_SIZES = [8, 8, 8, 4, 2, 1, 1]         # chunks per input DMA (sum = 32)
OUT_REP = 8             # row-chunks per output DMA -> 4 MB writes
N_OUT = N_CHUNK // OUT_REP
H = 512                 # PSUM bank width in f32 (matmul N limit)

_CACHE: dict = {}


def _build_program():
    import concourse.mybir as mybir
    import concourse.tile as tile
    from concourse import bacc

    assert sum(IN_SIZES) == N_CHUNK
    f32 = mybir.dt.float32
    nc = bacc.Bacc("TRN2", target_bir_lowering=False, debug=False, num_devices=B, enable_asserts=False)
    v = nc.declare_dram_parameter("value", [S, D], f32, isOutput=False)
    o = nc.declare_dram_parameter("out", [S, D], f32, isOutput=True)

    v_rows = v[:].rearrange("(c p) m -> c p m", p=P)       # [32][128][1024]
    o_re = o[:].rearrange("(i n p) m -> i p n m", i=N_OUT, n=OUT_REP, p=P)

    # Per-chunk reduction cost: DVE tensor_add ~1.23 us; PE (f32 matmul,
    # 2 HW passes per N=512 bank) ~2.2 us. Balance ~2:1 DVE:PE so both
    # trail the DMA stream. The last chunks go to the PE with the DVE-
    # accumulator fold emitted before them in PE queue order, so the
    # critical tail after the last input byte is just 2 PE passes + copy.
    pe_chunks = {c for c in range(N_CHUNK) if c % 3 == 2 and c < N_CHUNK - 2}
    pe_chunks |= {N_CHUNK - 2, N_CHUNK - 1}
    first_pe = min(pe_chunks)
    last_pe = N_CHUNK - 1
    fold_after = max(c for c in range(N_CHUNK) if c not in pe_chunks)  # last DVE chunk

    with tile.TileContext(nc) as tc:
        with (
            tc.tile_pool(name="in", bufs=1) as in_pool,
            tc.tile_pool(name="acc", bufs=1) as acc_pool,
            tc.tile_pool(name="ones", bufs=1) as ones_pool,
            tc.tile_pool(name="bcast", bufs=1) as bcast_pool,
            tc.tile_pool(name="psum", bufs=1, space="PSUM") as psum_pool,
        ):
            ones = ones_pool.tile([P, P], f32)
            nc.vector.memset(ones[:], 1.0)

            ps = psum_pool.tile([P, D], f32)
            acc = acc_pool.tile([P, D], f32)
            chunk0 = 0
            n_dve = 0
            for ti, sz in enumerate(IN_SIZES):
                t = in_pool.tile([P, sz * D], f32, tag=f"in{ti}")
                # DRAM side: rows [chunk0*128, (chunk0+sz)*128)
                src = v_rows[chunk0 : chunk0 + sz].rearrange("n p m -> p n m")
                nc.sync.dma_start(t[:].rearrange("p (n m) -> p n m", n=sz), src)
                for n in range(sz):
                    c = chunk0 + n
                    sl = t[:, n * D : (n + 1) * D]
                    if c in pe_chunks:
                        for h in range(2):
                            nc.tensor.matmul(
                                ps[:, h * H : (h + 1) * H],
                                ones[:],
                                sl[:, h * H : (h + 1) * H],
                                start=(c == first_pe),
                                stop=(c == last_pe),
                            )
                    elif n_dve == 0:
                        nc.vector.tensor_copy(acc[:], sl)
                        n_dve += 1
                    else:
                        nc.vector.tensor_add(acc[:], acc[:], sl)
                        n_dve += 1
                    if c == fold_after:
                        # Fold the DVE accumulator into PSUM (mid-group).
                        for h in range(2):
                            nc.tensor.matmul(
                                ps[:, h * H : (h + 1) * H],
                                ones[:],
                                acc[:, h * H : (h + 1) * H],
                                start=False,
                                stop=False,
                            )
                chunk0 += sz

            # PSUM -> SBUF in parallel halves (DVE + ACT) to shorten the tail.
            bc = bcast_pool.tile([P, D], f32)
            nc.vector.tensor_copy(bc[:, 0:H], ps[:, 0:H])
            nc.scalar.copy(bc[:, H:D], ps[:, H:D])

            src = bc[:].unsqueeze(1).to_broadcast((P, OUT_REP, D))
            for i in range(N_OUT):
                nc.sync.dma_start(o_re[i], src)

    nc.compile()
    return nc


def _get_program():
    if "nc" not in _CACHE:
        _CACHE["nc"] = _build_program()
    return _CACHE["nc"]


def kernel(query: np.ndarray, value: np.ndarray) -> np.ndarray:
    from concourse.bass_utils import run_bass_kernel_spmd

    del query  # output is exactly independent of query (see module docstring)
    value = np.ascontiguousarray(value, dtype=np.float32)
    assert value.shape == (B, S, D)

    nc = _get_program()
    in_maps = [{"value": value[b]} for b in range(B)]
    try:
        res = run_bass_kernel_spmd(nc, in_maps, list(range(B)))
    except Exception:
        # The tunneled runtime occasionally surfaces a transient
        # NRT_EXEC_UNIT_UNRECOVERABLE on the first dispatch; retry once.
        import time

        time.sleep(2.0)
        res = run_bass_kernel_spmd(nc, in_maps, list(range(B)))
    return np.stack([res.results[b]["out"] for b in range(B)], axis=0)



# revision 3
# speedup vs baseline: 1.4984x; 1.4984x over previous
"""Trainium2 Bass kernel for nn_CompressiveMemory_57750130262084.

The reference computes (B=8, S=4096, DK=DV=1024):
    sigma  = elu(query) + 1                                  [B,S,DK]
    memory = einsum('bkd,bsv->bkv', swap(sigma), value)      [B,DK,DV]
    z_norm = sum_s sigma                                     [B,DK]
    out    = einsum('bsd,bkv->bsv', sigma, memory)
           / einsum('bsd,bk->bs',  sigma, z_norm)[..., None]

Every einsum uses disjoint summed subscripts, so each factorises into
outer products of independent reductions:
    memory[b,k,v]    = z_norm[b,k] * VS[b,v]      with VS[b,v] = sum_s value[b,s,v]
    retrieved[b,s,v] = rs[b,s] * Z[b] * VS[b,v]   with rs = rowsum(sigma), Z = sum_k z_norm
    denom[b,s]       = rs[b,s] * Z[b]
    out[b,s,v]       = VS[b,v]                    (exactly; query cancels)

So the kernel is a column-sum of `value` over S, broadcast over S.
Sharding: data-parallel over batch, one NeuronCore per batch element.

Schedule per core (v2):
  - p-major input layout: partition p holds 32 CONTIGUOUS DRAM rows
    [32p, 32p+32) so input DMA descriptors can be up to 128 KB (vs the
    4 KB a row-major layout allows).  Which rows land on which
    partition is irrelevant: everything gets summed.
  - input split across BOTH HWDGE engines (SP + Activation), rows
    0..15 / 16..31, transfers of [8,4,2,1,1] rows each descending so
    the completion tail is fine-grained.
  - per-[128,1024] chunk reduction split between DVE (fp32 tensor_add
    chain into acc) and PE (PSUM-accumulating ones^T @ chunk, which
    partition-reduces AND broadcasts).  The DVE acc is folded into
    PSUM by a final ones^T @ acc with stop=True.
  - output stored as float16 (tolerance is 2e-2; fp16 adds ~1.6e-4),
    halving write traffic.  PSUM -> SBUF conversion copies run on DVE
    and ACT in parallel writing two replicas, so output descriptors
    are 4 KB; ACT's table load is pre-warmed at t~0.  Output DMAs
    alternate between the two HWDGE engines; host upcasts to f32.
"""

import numpy as np

B, S, D = 8, 4096, 1024
P = 128                 # SBUF partitions
RPP = S // P            # 32 rows per partition (p-major layout)
N_CHUNK = 32            # [128,1024] column chunks of the SBUF tile
GROUPS = [8, 4, 2, 1, 1]  # rows/partition per input transfer (per engine)
REP = 2                 # output row-replicas in SBUF -> 4KB descriptors
N_OUT = 16              # output transfers (256 rows each)
H = 512                 # PSUM bank width in f32 (matmul N limit)

_CACHE: dict = {}


def _build_program():
    import concourse.mybir as mybir
    import concourse.tile as tile
    from concourse import bacc

    f32 = mybir.dt.float32
    f16 = mybir.dt.float16
    assert sum(GROUPS) == 16
    nc = bacc.Bacc("TRN2", target_bir_lowering=False, debug=False, num_devices=B, enable_asserts=False)
    v = nc.declare_dram_parameter("value", [S, D], f32, isOutput=False)
    o = nc.declare_dram_parameter("out", [S, D], f16, isOutput=True)

    v_pm = v[:].rearrange("(p r) m -> p (r m)", p=P)       # [128][32*1024]
    o_re = o[:].rearrange("(i p n) m -> i p (n m)", p=P, n=REP)  # [16][128][2048]

    # Arrival-ordered chunk list: transfers complete pairwise (one per
    # engine), sync rows 0..15 = chunks 0..15, scalar rows 16..31 =
    # chunks 16..31, groups [8,4,2,1,1] per engine.
    order = (
        list(range(0, 8)) + list(range(16, 24))      # s0, a0 (4 MB each)
        + list(range(8, 12)) + list(range(24, 28))   # s1, a1 (2 MB)
        + [12, 13] + [28, 29]                        # s2, a2 (1 MB)
        + [14] + [30]                                # s3, a3 (0.5 MB)
        + [15] + [31]                                # s4, a4 (0.5 MB)
    )
    # PE chunks: spread through the stream; c15 and c30 at the tail so
    # the last DVE add (c31) overlaps PE's final chunk work, then the
    # fold is PE's last op.
    pe_pos = {2, 4, 7, 10, 12, 15, 18, 20, 23, 26, 29, 30}
    pe_chunks = {order[i] for i in pe_pos}
    first_pe = order[min(pe_pos)]
    dve_chunks = [order[i] for i in range(N_CHUNK) if i not in pe_pos]

    with tile.TileContext(nc) as tc:
        with (
            tc.tile_pool(name="in", bufs=1) as in_pool,
            tc.tile_pool(name="acc", bufs=1) as acc_pool,
            tc.tile_pool(name="ones", bufs=1) as ones_pool,
            tc.tile_pool(name="bcast", bufs=1) as bcast_pool,
            tc.tile_pool(name="warm", bufs=1) as warm_pool,
            tc.tile_pool(name="psum", bufs=1, space="PSUM") as psum_pool,
        ):
            ones = ones_pool.tile([P, P], f32)
            nc.vector.memset(ones[:], 1.0)
            warm = warm_pool.tile([P, 16], f32)
            nc.scalar.copy(warm[:], ones[:, 0:16])   # pre-warm ACT table load

            t = in_pool.tile([P, RPP * D], f32)
            ps = psum_pool.tile([P, D], f32)
            acc = acc_pool.tile([P, D], f32)

            # Input DMAs: each engine issues its transfers back-to-back.
            for half, eng in ((0, nc.sync), (1, nc.scalar)):
                r0 = half * 16
                for g in GROUPS:
                    sl = slice(r0 * D, (r0 + g) * D)
                    eng.dma_start(t[:, sl], v_pm[:, sl])
                    r0 += g

            # Reduction, issued in expected arrival order.
            n_dve = 0
            for c in order:
                sl = t[:, c * D : (c + 1) * D]
                if c in pe_chunks:
                    for h in range(2):
                        nc.tensor.matmul(
                            ps[:, h * H : (h + 1) * H],
                            ones[:],
                            sl[:, h * H : (h + 1) * H],
                            start=(c == first_pe),
                            stop=False,
                        )
                elif n_dve == 0:
                    nc.vector.tensor_copy(acc[:], sl)
                    n_dve += 1
                else:
                    nc.vector.tensor_add(acc[:], acc[:], sl)
                    n_dve += 1
            # Fold the DVE accumulator into PSUM; closes the group.
            for h in range(2):
                nc.tensor.matmul(
                    ps[:, h * H : (h + 1) * H],
                    ones[:],
                    acc[:, h * H : (h + 1) * H],
                    start=False,
                    stop=(h == 1),
                )

            # PSUM -> SBUF f16 conversion, two replicas in parallel
            # (DVE + ACT) so output descriptors cover REP=2 rows (4KB).
            bc = bcast_pool.tile([P, REP * D], f16)
            nc.vector.tensor_copy(bc[:, 0:D], ps[:])
            nc.scalar.copy(bc[:, D : 2 * D], ps[:])

            for i in range(N_OUT):
                eng = nc.sync if i % 2 == 0 else nc.scalar
                eng.dma_start(o_re[i], bc[:])

    nc.compile()
    return nc


def _get_program():
    if "nc" not in _CACHE:
        _CACHE["nc"] = _build_program()
    return _CACHE["nc"]


def kernel(query: np.ndarray, value: np.ndarray) -> np.ndarray:
    from concourse.bass_utils import run_bass_kernel_spmd

    del query  # output is exactly independent of query (see module docstring)
    value = np.ascontiguousarray(value, dtype=np.float32)
    assert value.shape == (B, S, D)

    nc = _get_program()
    in_maps = [{"value": value[b]} for b in range(B)]
    try:
        res = run_bass_kernel_spmd(nc, in_maps, list(range(B)))
    except Exception:
        # The tunneled runtime occasionally surfaces a transient
        # NRT_EXEC_UNIT_UNRECOVERABLE on the first dispatch; retry once.
        import time

        time.sleep(2.0)
        res = run_bass_kernel_spmd(nc, in_maps, list(range(B)))
    return np.stack(
        [res.results[b]["out"].astype(np.float32) for b in range(B)], axis=0
    )
